# revision 6
# baseline (speedup 1.0000x reference)
"""CompactBilinearPooling kernel for Trainium2 (8 NeuronCores, SPMD data-parallel).

Per core (32 batch rows):
  1. Count-sketch both inputs into one fused DRAM table Y[bin] = [y1 | y2]
     (64 f32 per row). The d-axis is rank-sorted on the HOST (stable sort by
     collision rank, signs s1 pre-applied, columns permuted/padded to
     128-blocks), so scatter round r reads a contiguous block-slice of the
     PE-transposed data and carries ONLY real indices (num_idxs_reg = n_r,
     trailing -1 padding). Rounds use prepare_only SWDGE descriptor
     generation alternated with trigger_dma so Q7 gen overlaps the
     transposes and the previous round's DMA drain. Round 0 (~87% of
     entries, collision-free) is split into sub-chunks triggered as their
     transposed blocks land.
  2. Circular convolution via FFT packing trick: Z = FFT(y1 + i*y2),
     out = Im(IFFT(Z^2))/2. Length-16384 FFT = 128x128 four-step with DFT-128
     matmuls on the PE in float32r. Twiddle complex products + recombines run
     on DVE straight out of PSUM; squares/copies on ACT.
"""
import sys

sys.path.insert(0, "/opt/trn_rl_repo")

import numpy as np

import concourse.bass as bass
import concourse.bacc as bacc
import concourse.mybir as mybir
import concourse.tile as tile
from concourse.bass_utils import run_bass_kernel_spmd
P = 128
B, D, O = 256, 4096, 16384
NCORES = 8
BC = B // NCORES          # 32 rows per core
F32R = mybir.dt.float32r
F32 = mybir.dt.float32

_cache = {}
PER_GROUP_OUT = True
SUBCHUNK = 8              # blocks per scatter sub-chunk of round 0
PREP = False              # prepare_only + trigger_dma (broken on HW?)


def _build(seg_counts: tuple, skip_scatter=False, skip_fft=False):
    """seg_counts[r] = number of real indices in scatter round r."""
    nblks = [(n + P - 1) // P for n in seg_counts]
    T = sum(nblks) * P            # padded d-axis length
    NTs = T // P                  # transpose chunks
    nc = bacc.Bacc("TRN2", target_bir_lowering=False, debug=False)

    # ---- I/O ----
    x1c = nc.dram_tensor("x1c", [BC, T], F32R, kind="ExternalInput")
    x2c = nc.dram_tensor("x2c", [BC, T], F32R, kind="ExternalInput")
    idxs = nc.dram_tensor("idxs", [P, T // 16], mybir.dt.int16, kind="ExternalInput")
    wa1 = nc.dram_tensor("wa1", [P, 2 * P], F32R, kind="ExternalInput")    # [WFre | WFim]
    wa2 = nc.dram_tensor("wa2", [P, 2 * P], F32R, kind="ExternalInput")    # [-WFim | WFre]
    wfre = nc.dram_tensor("wfre", [P, P], F32R, kind="ExternalInput")
    wfim = nc.dram_tensor("wfim", [P, P], F32R, kind="ExternalInput")
    wfimn = nc.dram_tensor("wfimn", [P, P], F32R, kind="ExternalInput")    # -WFim
    wi1 = nc.dram_tensor("wi1", [P, 2 * P], F32R, kind="ExternalInput")    # [WIre | WIim]
    wi2 = nc.dram_tensor("wi2", [P, 2 * P], F32R, kind="ExternalInput")    # [-2WIim | 2WIre]
    wire = nc.dram_tensor("wire", [P, P], F32R, kind="ExternalInput")
    wiim = nc.dram_tensor("wiim", [P, P], F32R, kind="ExternalInput")
    t1re = nc.dram_tensor("t1re", [P, 4 * P], F32R, kind="ExternalInput")   # bcast over 4 rows
    t1im = nc.dram_tensor("t1im", [P, 4 * P], F32R, kind="ExternalInput")
    t1imn = nc.dram_tensor("t1imn", [P, 4 * P], F32R, kind="ExternalInput")
    t2re = nc.dram_tensor("t2re", [P, 4 * P], F32R, kind="ExternalInput")   # x 1/(2N)
    t2im = nc.dram_tensor("t2im", [P, 4 * P], F32R, kind="ExternalInput")
    t2imn = nc.dram_tensor("t2imn", [P, 4 * P], F32R, kind="ExternalInput")
    identm = nc.dram_tensor("identm", [BC, BC], F32R, kind="ExternalInput")
    out = nc.dram_tensor("out", [BC, O], F32, kind="ExternalOutput")

    with tile.TileContext(nc) as tc:
        with (
            tc.tile_pool(name="const", bufs=1) as cp,
            tc.tile_pool(name="work", bufs=1) as wp,
            tc.tile_pool(name="tmp", bufs=2) as tp,
            tc.tile_pool(name="psum", bufs=4, space="PSUM") as pp,
            tc.tile_pool(name="dram", bufs=1, space="DRAM") as dp,
        ):
            # ---- fused sketch table in DRAM: row = [y1(32) | y2(32)] ----
            yd = dp.tile([O, 64], F32R)

            # Fast path on the sync HWDGE ring: ident, idxs, x1, x2.
            ident_t = cp.tile([BC, BC], F32R, tag="identm")
            nc.sync.dma_start(ident_t[:], identm[:])
            idxs_s = cp.tile([P, T // 16], mybir.dt.int16)
            nc.sync.dma_start(idxs_s[:], idxs[:])
            xs1 = wp.tile([BC, T], F32R, tag="xs1_y")
            xs2 = wp.tile([BC, T], F32R, tag="xs2_s")
            nc.sync.dma_start(xs1[:], x1c[:])
            nc.sync.dma_start(xs2[:], x2c[:])

            # Zero-init the DRAM table on the scalar HWDGE ring (parallel
            # with the sync-ring loads above).
            zsb = wp.tile([P, 4096], F32, tag="zero_osb")
            nc.vector.memset(zsb[:], 0.0)
            ydv = yd[:].rearrange("(h p a) e -> h p a e", h=2, p=P)
            for h in range(2):
                nc.scalar.dma_start(ydv[h], zsb[:].bitcast(F32R).rearrange("p (a e) -> p a e", e=64))

            def cload(src, shape, dtype=F32R):
                t = cp.tile(shape, dtype, tag=src.name)
                nc.scalar.dma_start(t[:], src[:])
                return t

            # FFT constants on the scalar ring: only needed ~40us in.
            wa1_s = cload(wa1, [P, 2 * P])
            wa2_s = cload(wa2, [P, 2 * P])
            wfre_s = cload(wfre, [P, P])
            wfim_s = cload(wfim, [P, P])
            wfimn_s = cload(wfimn, [P, P])
            wi1_s = cload(wi1, [P, 2 * P])
            wi2_s = cload(wi2, [P, 2 * P])
            wire_s = cload(wire, [P, P])
            wiim_s = cload(wiim, [P, P])
            t1re_s = cload(t1re, [P, 4 * P])
            t1im_s = cload(t1im, [P, 4 * P])
            t1imn_s = cload(t1imn, [P, 4 * P])
            t2re_s = cload(t2re, [P, 4 * P])
            t2im_s = cload(t2im, [P, 4 * P])
            t2imn_s = cload(t2imn, [P, 4 * P])
            ident = ident_t[:]

            # ---- transpose x (signs pre-applied on host) -> sxT, in
            # 8-block PSUM groups so early scatter sub-chunks can trigger
            # as soon as their blocks are evacuated.
            sxT = wp.tile([P, NTs * 64], F32R, tag="sxT")
            ngroups = (NTs + SUBCHUNK - 1) // SUBCHUNK
            for g in range(ngroups):
                lo = g * SUBCHUNK
                hi = min(lo + SUBCHUNK, NTs)
                ps = pp.tile([P, 1024], F32R, space="PSUM", tag="ps")
                for jj in range(hi - lo):
                    j = lo + jj
                    nc.tensor.transpose(out=ps[:, jj * 64:jj * 64 + 32],
                                        in_=xs1[:, j * P:(j + 1) * P], identity=ident)
                    nc.tensor.transpose(out=ps[:, jj * 64 + 32:jj * 64 + 64],
                                        in_=xs2[:, j * P:(j + 1) * P], identity=ident)
                nc.vector.tensor_copy(sxT[:, lo * 64:hi * 64], ps[:, :(hi - lo) * 64])

            # ---- scatter: prepare_only desc-gen alternated with triggers.
            # Sub-chunks: round 0 split into SUBCHUNK-block pieces (all
            # collision-free, but triggered separately so drains overlap
            # later gen); rounds 1+ one piece each.
            inap_full = sxT[:].rearrange("p (t e) -> p t e", e=64)
            pieces = []  # (blk0, nblk, n_real)
            blk0 = 0
            for r, (n_r, nblk) in enumerate(zip(seg_counts, nblks)):
                if r == 0:
                    left, b0 = n_r, blk0
                    while left > 0:
                        nb = min(SUBCHUNK, nblk - (b0 - blk0))
                        take = min(left, nb * P)
                        pieces.append((b0, nb, take))
                        b0 += nb
                        left -= take
                else:
                    pieces.append((blk0, nblk, n_r))
                blk0 += nblk

            if not skip_scatter:
                for i, (b0, nb, n_real) in enumerate(pieces):
                    if PREP:
                        sem = nc.alloc_semaphore(f"scat_dma_{i}")
                        nc.gpsimd.dma_scatter_add(
                            out_ap=yd[:],
                            in_ap=inap_full[:, b0:b0 + nb, :],
                            idxs_ap=idxs_s[:, b0 * 8:(b0 + nb) * 8],
                            num_idxs=nb * P,
                            num_idxs_reg=n_real,
                            elem_size=64,
                            prepare_only=True,
                            sem=sem,
                        )
                        nc.gpsimd.trigger_dma(count=None)
                    else:
                        nc.gpsimd.dma_scatter_add(
                            out_ap=yd[:],
                            in_ap=inap_full[:, b0:b0 + nb, :],
                            idxs_ap=idxs_s[:, b0 * 8:(b0 + nb) * 8],
                            num_idxs=nb * P,
                            num_idxs_reg=n_real,
                            elem_size=64,
                        )

            # ---- reload fused sketch as [q, (n2, 64)] ----
            yf = wp.tile([P, P * 64], F32R, tag="xs1_y")
            nc.sync.dma_start(yf[:].rearrange("q (n e) -> q n e", e=64),
                              yd[0:O, :].rearrange("(q n) e -> q n e", q=P))
            yf_r = yf[:].rearrange("q (n e) -> q n e", e=64)

            r3 = lambda ap: ap.rearrange("p (b2 k) -> p b2 k", b2=4)

            # ---- FFT: software-pipelined across 4-row groups ----
            ssb_re = wp.tile([P, P * BC], F32R, tag="ssb_re")
            ssb_im = wp.tile([P, P * BC], F32R, tag="ssb_im")
            osb = wp.tile([P, P * BC], F32, tag="zero_osb")
            mt, nt_ = {}, {}

            def stage_a(g):
                ps = pp.tile([P, 1024], F32, space="PSUM", tag="ps")
                for bb in range(4):
                    b_ = g * 4 + bb
                    sl = ps[:, bb * 256:(bb + 1) * 256]
                    nc.tensor.matmul(out=sl, lhsT=yf_r[:, :, b_], rhs=wa1_s[:], start=True, stop=False)
                    nc.tensor.matmul(out=sl, lhsT=yf_r[:, :, 32 + b_], rhs=wa2_s[:], start=False, stop=True)
                pre = ps[:].rearrange("p (b2 h k) -> p b2 h k", b2=4, h=2)[:, :, 0, :]
                pim = ps[:].rearrange("p (b2 h k) -> p b2 h k", b2=4, h=2)[:, :, 1, :]
                m1 = tp.tile([P, 512], F32R, tag="m1")
                m2 = tp.tile([P, 512], F32R, tag="m2")
                mA = tp.tile([P, 512], F32R, tag="mA")
                mB = tp.tile([P, 512], F32R, tag="mB")
                # mA = pre*t1re - pim*t1im ; mB = pre*t1im + pim*t1re
                nc.vector.tensor_mul(r3(m1[:]), pre, r3(t1re_s[:]))
                nc.vector.tensor_mul(r3(m2[:]), pim, r3(t1imn_s[:]))
                nc.vector.tensor_add(mA[:], m1[:], m2[:])
                nc.vector.tensor_mul(r3(m1[:]), pre, r3(t1im_s[:]))
                nc.vector.tensor_mul(r3(m2[:]), pim, r3(t1re_s[:]))
                nc.vector.tensor_add(mB[:], m1[:], m2[:])
                mt[g] = (mA, mB)

            def stage_b(g):
                mA, mB = mt.pop(g)
                rs = slice(g * 512, (g + 1) * 512)
                ps = pp.tile([P, 1024], F32, space="PSUM", tag="ps")
                zre, zim = ps[:, 0:512], ps[:, 512:1024]
                nc.tensor.matmul(out=zre, lhsT=wfre_s[:], rhs=mA[:], start=True, stop=False)
                nc.tensor.matmul(out=zre, lhsT=wfimn_s[:], rhs=mB[:], start=False, stop=True)
                nc.tensor.matmul(out=zim, lhsT=wfim_s[:], rhs=mA[:], start=True, stop=False)
                nc.tensor.matmul(out=zim, lhsT=wfre_s[:], rhs=mB[:], start=False, stop=True)
                u = tp.tile([P, 512], F32R, tag="u")
                v = tp.tile([P, 512], F32R, tag="v")
                w_ = tp.tile([P, 512], F32R, tag="w")
                nc.scalar.activation(u[:], zre, mybir.ActivationFunctionType.Square)
                nc.scalar.activation(v[:], zim, mybir.ActivationFunctionType.Square)
                nc.scalar.copy(w_[:], zim)
                nc.vector.tensor_sub(ssb_re[:, rs], u[:], v[:])
                nc.vector.tensor_mul(ssb_im[:, rs], zre, w_[:])

            def stage_c(g):
                ps = pp.tile([P, 1024], F32, space="PSUM", tag="ps")
                for bb in range(4):
                    b_ = g * 4 + bb
                    sl = ps[:, bb * 256:(bb + 1) * 256]
                    lre = ssb_re[:, b_ * P:(b_ + 1) * P]
                    lim = ssb_im[:, b_ * P:(b_ + 1) * P]
                    nc.tensor.matmul(out=sl, lhsT=lre, rhs=wi1_s[:], start=True, stop=False)
                    nc.tensor.matmul(out=sl, lhsT=lim, rhs=wi2_s[:], start=False, stop=True)
                preC = ps[:].rearrange("p (b2 h k) -> p b2 h k", b2=4, h=2)[:, :, 0, :]
                pimC = ps[:].rearrange("p (b2 h k) -> p b2 h k", b2=4, h=2)[:, :, 1, :]
                n1 = tp.tile([P, 512], F32R, tag="n1")
                n2 = tp.tile([P, 512], F32R, tag="n2")
                nA = tp.tile([P, 512], F32R, tag="nA")
                nB = tp.tile([P, 512], F32R, tag="nB")
                # nA = preC*t2re - pimC*t2im ; nB = preC*t2im + pimC*t2re
                nc.vector.tensor_mul(r3(n1[:]), preC, r3(t2re_s[:]))
                nc.vector.tensor_mul(r3(n2[:]), pimC, r3(t2imn_s[:]))
                nc.vector.tensor_add(nA[:], n1[:], n2[:])
                nc.vector.tensor_mul(r3(n1[:]), preC, r3(t2im_s[:]))
                nc.vector.tensor_mul(r3(n2[:]), pimC, r3(t2re_s[:]))
                nc.vector.tensor_add(nB[:], n1[:], n2[:])
                nt_[g] = (nA, nB)

            def stage_d(g):
                nA, nB = nt_.pop(g)
                rs = slice(g * 512, (g + 1) * 512)
                ps = pp.tile([P, 1024], F32, space="PSUM", tag="ps")
                po = ps[:, 0:512]
                nc.tensor.matmul(out=po, lhsT=wiim_s[:], rhs=nA[:], start=True, stop=False)
                nc.tensor.matmul(out=po, lhsT=wire_s[:], rhs=nB[:], start=False, stop=True)
                nc.scalar.copy(osb[:, rs], po)
                if PER_GROUP_OUT:
                    nc.sync.dma_start(
                        out[:].rearrange("b (a c) -> a b c", c=P)[:, g * 4:(g + 1) * 4, :],
                        osb[:, rs].rearrange("a (b c) -> a b c", c=P))

            for gg in range(11):
                if gg < 8 and not skip_fft:
                    stage_a(gg)
                if 1 <= gg < 9 and not skip_fft:
                    stage_b(gg - 1)
                if 2 <= gg < 10 and not skip_fft:
                    stage_c(gg - 2)
                if 3 <= gg and not skip_fft:
                    stage_d(gg - 3)
            if skip_fft:
                nc.vector.memset(osb[:], 0.0)
            if not PER_GROUP_OUT or skip_fft:
                nc.sync.dma_start(out[:].rearrange("b (a c) -> a b c", c=P),
                                  osb[:].rearrange("a (b c) -> a b c", c=P))

    nc.compile()
    return nc


def _host_consts():
    j = np.arange(P)
    f32 = np.float32
    ang = -2.0 * np.pi * np.outer(j, j) / P
    wf_re, wf_im = np.cos(ang), np.sin(ang)
    wi_re, wi_im = np.cos(-ang), np.sin(-ang)
    wa1 = np.concatenate([wf_re, wf_im], axis=1).astype(f32)
    wa2 = np.concatenate([-wf_im, wf_re], axis=1).astype(f32)
    wi1 = np.concatenate([wi_re, wi_im], axis=1).astype(f32)
    wi2 = np.concatenate([-2.0 * wi_im, 2.0 * wi_re], axis=1).astype(f32)
    tang = -2.0 * np.pi * np.outer(j, j) / O
    t1re_1 = np.cos(tang)
    t1im_1 = np.sin(tang)
    scale = 1.0 / (2.0 * O)
    t2re_1 = np.cos(tang) * scale      # cos(+x) = cos(-x)
    t2im_1 = -np.sin(tang) * scale     # sin(+x) = -sin(-x)

    def b4(m):
        return np.tile(m[:, None, :], (1, 4, 1)).reshape(P, 4 * P).astype(f32)

    return dict(
        wa1=wa1, wa2=wa2, wi1=wi1, wi2=wi2,
        wfre=wf_re.astype(f32), wfim=wf_im.astype(f32), wfimn=(-wf_im).astype(f32),
        wire=wi_re.astype(f32), wiim=wi_im.astype(f32),
        t1re=b4(t1re_1), t1im=b4(t1im_1), t1imn=b4(-t1im_1),
        t2re=b4(t2re_1), t2im=b4(t2im_1), t2imn=b4(-t2im_1),
        identm=np.eye(BC, dtype=f32),
    )


def _host_prep(h1, s1):
    """Rank-sort the d-axis: permutation (by collision rank), per-round
    segment counts, padded int16 index table in wrapped layout."""
    h1 = np.asarray(h1, dtype=np.int64)
    s1 = np.asarray(s1, dtype=np.float32)
    rank = np.zeros(D, np.int64)
    seen = {}
    for d in range(D):
        b = int(h1[d])
        rank[d] = seen.get(b, 0)
        seen[b] = int(rank[d]) + 1
    n_rounds = int(rank.max()) + 1
    perm_parts = [np.where(rank == r)[0] for r in range(n_rounds)]
    seg_counts = tuple(int(p.size) for p in perm_parts)
    nblks = [(n + P - 1) // P for n in seg_counts]
    T = sum(nblks) * P
    # padded position -> original d (or -1 for pad)
    pos2d = np.full(T, -1, np.int64)
    flat_idx = np.full(T, -1, np.int64)
    blk0 = 0
    for r, part in enumerate(perm_parts):
        n_r = part.size
        pos2d[blk0 * P:blk0 * P + n_r] = part
        flat_idx[blk0 * P:blk0 * P + n_r] = h1[part]
        blk0 += nblks[r]
    # round-0 sub-chunks have their own -1 tails within the segment:
    # positions beyond each sub-chunk's num_idxs_reg must be -1 ONLY at the
    # very end; sub-chunks split a dense prefix, so interior sub-chunks are
    # full (num_idxs_reg == nb*P) and only the last partial one has a tail.
    # wrapped int16 index table: position i at [i%16, i//16], replicated x8
    wrapped = flat_idx.astype(np.int16).reshape(T // 16, 16).T
    idxs = np.tile(wrapped, (8, 1))
    return seg_counts, pos2d, idxs


_last_results = None


def kernel(x1, x2, h1, s1, output_size=O, **kw):
    global _last_results
    x1 = np.asarray(x1, np.float32)
    x2 = np.asarray(x2, np.float32)
    s1 = np.asarray(s1, np.float32)
    seg_counts, pos2d, idxs = _host_prep(h1, s1)
    T = pos2d.size
    # permuted, sign-scaled, zero-padded inputs
    sx1 = np.zeros((B, T), np.float32)
    sx2 = np.zeros((B, T), np.float32)
    valid = pos2d >= 0
    sx1[:, valid] = x1[:, pos2d[valid]] * s1[pos2d[valid]]
    sx2[:, valid] = x2[:, pos2d[valid]] * s1[pos2d[valid]]
    if seg_counts not in _cache:
        _cache[seg_counts] = _build(seg_counts)
    nc = _cache[seg_counts]
    consts = _host_consts()
    in_maps = []
    for c in range(NCORES):
        m = dict(consts)
        m["x1c"] = sx1[c * BC:(c + 1) * BC]
        m["x2c"] = sx2[c * BC:(c + 1) * BC]
        m["idxs"] = idxs
        in_maps.append(m)
    res = run_bass_kernel_spmd(nc, in_maps, core_ids=list(range(NCORES)))
    _last_results = res
    return np.concatenate([res.results[c]["out"] for c in range(NCORES)], axis=0)


# revision 7
# speedup vs baseline: 2.1458x; 2.1458x over previous
"""CompactBilinearPooling kernel for Trainium2 — one-hot matmul binning.

Per core (32 batch rows):
  1. Count-sketch via PER-GROUP ONE-HOT MATMULS instead of DMA scatter-add:
     host groups features d by qlow = h1[d] % 128 (the table's free digit),
     pads each group to 64 slots, pre-applies signs, and uploads a
     block-diagonal slot tensor sxE2 [128, 64*128] where the q-pair
     (2*qq, 2*qq+1) occupies partitions [0:64)/[64:128) and batch columns
     [0:64)/[64:128) of block qq. The device builds one-hot matrices
     N[slot, nhigh] (nhigh = h1[d] // 128) via is_equal against an iota
     table, then one matmul per q-pair:
        Y[nhigh, (q, b)] += N^T @ sxE2-block
     Duplicate bins sum natively in the contraction; untouched bins get
     exact zeros from start=True. The sketch table lands directly in SBUF
     in the FFT stage-1 layout (partition digit = bin//128, free digit =
     bin%128) — no DRAM table, no zero-init, no scatter, no reload.
  2. Circular convolution via FFT packing trick: Z = FFT(y1 + i*y2),
     out = Im(IFFT(Z^2))/2. Length-16384 FFT = 128x128 four-step with
     DFT-128 matmuls on the PE in float32r. Twiddle complex products +
     recombines on DVE straight out of PSUM; squares/copies on ACT;
     two products per group on GPSIMD via ACT relay copies.
"""
import sys

sys.path.insert(0, "/opt/trn_rl_repo")

import numpy as np

import concourse.bass as bass
import concourse.bacc as bacc
import concourse.mybir as mybir
import concourse.tile as tile
from concourse.bass_utils import run_bass_kernel_spmd
P = 128
B, D, O = 256, 4096, 16384
NCORES = 8
BC = B // NCORES          # 32 rows per core
F32R = mybir.dt.float32r
F32 = mybir.dt.float32

_cache = {}
PER_GROUP_OUT = True


def _build(slot_pad: int, skip_fft=False):
    """slot_pad: slots per q-group (64 normally; 128 fallback when some
    group exceeds 64 members)."""
    assert slot_pad == 64, "128-slot fallback not implemented; add if needed"
    NQQ = 64                  # q-pairs
    nc = bacc.Bacc("TRN2", target_bir_lowering=False, debug=False)

    # ---- I/O ----
    sxe = nc.dram_tensor("sxe", [P, NQQ * P], F32R, kind="ExternalInput")
    nhv = nc.dram_tensor("nhv", [P, NQQ], F32R, kind="ExternalInput")
    iot = nc.dram_tensor("iot", [P, 8 * P], F32R, kind="ExternalInput")
    wa1 = nc.dram_tensor("wa1", [P, 2 * P], F32R, kind="ExternalInput")    # [WFre | WFim]
    wa2 = nc.dram_tensor("wa2", [P, 2 * P], F32R, kind="ExternalInput")    # [-WFim | WFre]
    wfre = nc.dram_tensor("wfre", [P, P], F32R, kind="ExternalInput")
    wfim = nc.dram_tensor("wfim", [P, P], F32R, kind="ExternalInput")
    wfimn = nc.dram_tensor("wfimn", [P, P], F32R, kind="ExternalInput")    # -WFim
    wi1 = nc.dram_tensor("wi1", [P, 2 * P], F32R, kind="ExternalInput")    # [WIre | WIim]
    wi2 = nc.dram_tensor("wi2", [P, 2 * P], F32R, kind="ExternalInput")    # [-2WIim | 2WIre]
    wire = nc.dram_tensor("wire", [P, P], F32R, kind="ExternalInput")
    wiim = nc.dram_tensor("wiim", [P, P], F32R, kind="ExternalInput")
    t1re = nc.dram_tensor("t1re", [P, 4 * P], F32R, kind="ExternalInput")   # bcast over 4 rows
    t1im = nc.dram_tensor("t1im", [P, 4 * P], F32R, kind="ExternalInput")
    t1imn = nc.dram_tensor("t1imn", [P, 4 * P], F32R, kind="ExternalInput")
    t2re = nc.dram_tensor("t2re", [P, 4 * P], F32R, kind="ExternalInput")   # x 1/(2N)
    t2im = nc.dram_tensor("t2im", [P, 4 * P], F32R, kind="ExternalInput")
    t2imn = nc.dram_tensor("t2imn", [P, 4 * P], F32R, kind="ExternalInput")
    out = nc.dram_tensor("out", [BC, O], F32, kind="ExternalOutput")

    with tile.TileContext(nc) as tc:
        with (
            tc.tile_pool(name="const", bufs=1) as cp,
            tc.tile_pool(name="work", bufs=1) as wp,
            tc.tile_pool(name="tmp", bufs=2) as tp,
            tc.tile_pool(name="psum", bufs=4, space="PSUM") as pp,
        ):
            # Head loads, split across both HWDGE rings: nhv + iota first
            # (DVE one-hots), then sxE2 quarters alternating rings.
            nhv_s = cp.tile([P, NQQ], F32R, tag="nhv")
            nc.sync.dma_start(nhv_s[:], nhv[:])
            iot_s = cp.tile([P, 8 * P], F32R, tag="iot")
            nc.scalar.dma_start(iot_s[:], iot[:])
            sxe_s = wp.tile([P, NQQ * P], F32R, tag="sxe")
            sxe_v = sxe[:].rearrange("p (h c) -> h p c", h=4)
            sxs_v = sxe_s[:].rearrange("p (h c) -> h p c", h=4)
            for h in range(4):
                eng = nc.sync if h % 2 == 0 else nc.scalar
                eng.dma_start(sxs_v[h], sxe_v[h])

            def cload(src, shape, eng, dtype=F32R):
                t = cp.tile(shape, dtype, tag=src.name)
                eng.dma_start(t[:], src[:])
                return t

            wa1_s = cload(wa1, [P, 2 * P], nc.sync)
            wa2_s = cload(wa2, [P, 2 * P], nc.scalar)
            wfre_s = cload(wfre, [P, P], nc.sync)
            wfim_s = cload(wfim, [P, P], nc.scalar)
            wfimn_s = cload(wfimn, [P, P], nc.sync)
            wi1_s = cload(wi1, [P, 2 * P], nc.scalar)
            wi2_s = cload(wi2, [P, 2 * P], nc.sync)
            wire_s = cload(wire, [P, P], nc.scalar)
            wiim_s = cload(wiim, [P, P], nc.sync)
            t1re_s = cload(t1re, [P, 4 * P], nc.scalar)
            t1im_s = cload(t1im, [P, 4 * P], nc.sync)
            t1imn_s = cload(t1imn, [P, 4 * P], nc.scalar)
            t2re_s = cload(t2re, [P, 4 * P], nc.sync)
            t2im_s = cload(t2im, [P, 4 * P], nc.scalar)
            t2imn_s = cload(t2imn, [P, 4 * P], nc.sync)

            # ---- binning: 8 passes of 8 q-pairs each ----
            yf = wp.tile([P, P * 64], F32R, tag="yf")
            for qg in range(8):
                noh = tp.tile([P, 8 * P], F32R, tag="noh")
                nc.vector.tensor_tensor(
                    out=noh[:].rearrange("p (q t) -> p q t", q=8),
                    in0=nhv_s[:, qg * 8:(qg + 1) * 8].rearrange("p (q o) -> p q o", o=1).to_broadcast([P, 8, P]),
                    in1=iot_s[:].rearrange("p (q t) -> p q t", q=8),
                    op=mybir.AluOpType.is_equal,
                )
                ps = pp.tile([P, 1024], F32, space="PSUM", tag="ps")
                for i in range(8):
                    qq = qg * 8 + i
                    nc.tensor.matmul(
                        out=ps[:, i * P:(i + 1) * P],
                        lhsT=noh[:, i * P:(i + 1) * P],
                        rhs=sxe_s[:, qq * P:(qq + 1) * P],
                        start=True, stop=True,
                    )
                nc.vector.tensor_copy(yf[:, qg * 1024:(qg + 1) * 1024], ps[:])

            yf_r = yf[:].rearrange("q (n e) -> q n e", e=64)

            # ---- FFT: software-pipelined across 4-row groups ----
            ssb_re = wp.tile([P, P * BC], F32R, tag="ssb_re")
            ssb_im = wp.tile([P, P * BC], F32R, tag="ssb_im")
            osb = wp.tile([P, P * BC], F32, tag="osb")
            mt, nt_ = {}, {}

            r3 = lambda ap: ap.rearrange("p (b2 k) -> p b2 k", b2=4)

            def stage_a(g):
                ps = pp.tile([P, 1024], F32, space="PSUM", tag="ps")
                for bb in range(4):
                    b_ = g * 4 + bb
                    sl = ps[:, bb * 256:(bb + 1) * 256]
                    nc.tensor.matmul(out=sl, lhsT=yf_r[:, :, b_], rhs=wa1_s[:], start=True, stop=False)
                    nc.tensor.matmul(out=sl, lhsT=yf_r[:, :, 32 + b_], rhs=wa2_s[:], start=False, stop=True)
                pre = ps[:].rearrange("p (b2 h k) -> p b2 h k", b2=4, h=2)[:, :, 0, :]
                pim = ps[:].rearrange("p (b2 h k) -> p b2 h k", b2=4, h=2)[:, :, 1, :]
                m1 = tp.tile([P, 512], F32R, tag="m1")
                m2 = tp.tile([P, 512], F32R, tag="m2")
                m3 = tp.tile([P, 512], F32R, tag="m3")
                m4 = tp.tile([P, 512], F32R, tag="m4")
                mim = tp.tile([P, 512], F32R, tag="mim")
                nc.scalar.copy(mim[:], pim)  # ACT evac (GPSIMD cannot read PSUM)
                nc.vector.tensor_mul(r3(m1[:]), pre, r3(t1re_s[:]))
                nc.gpsimd.tensor_mul(r3(m2[:]), r3(mim[:]), r3(t1imn_s[:]))
                nc.vector.tensor_mul(r3(m3[:]), pre, r3(t1im_s[:]))
                nc.gpsimd.tensor_mul(r3(m4[:]), r3(mim[:]), r3(t1re_s[:]))
                mt[g] = (m1, m2, m3, m4)

            def stage_b(g):
                m1, m2, m3, m4 = mt.pop(g)
                rs = slice(g * 512, (g + 1) * 512)
                ps = pp.tile([P, 1024], F32, space="PSUM", tag="ps")
                zre, zim = ps[:, 0:512], ps[:, 512:1024]
                nc.tensor.matmul(out=zre, lhsT=wfre_s[:], rhs=m1[:], start=True, stop=False)
                nc.tensor.matmul(out=zre, lhsT=wfre_s[:], rhs=m2[:], start=False, stop=False)
                nc.tensor.matmul(out=zre, lhsT=wfimn_s[:], rhs=m3[:], start=False, stop=False)
                nc.tensor.matmul(out=zre, lhsT=wfimn_s[:], rhs=m4[:], start=False, stop=True)
                nc.tensor.matmul(out=zim, lhsT=wfim_s[:], rhs=m1[:], start=True, stop=False)
                nc.tensor.matmul(out=zim, lhsT=wfim_s[:], rhs=m2[:], start=False, stop=False)
                nc.tensor.matmul(out=zim, lhsT=wfre_s[:], rhs=m3[:], start=False, stop=False)
                nc.tensor.matmul(out=zim, lhsT=wfre_s[:], rhs=m4[:], start=False, stop=True)
                u = tp.tile([P, 512], F32R, tag="u")
                v = tp.tile([P, 512], F32R, tag="v")
                w_ = tp.tile([P, 512], F32R, tag="w")
                nc.scalar.activation(u[:], zre, mybir.ActivationFunctionType.Square)
                nc.scalar.activation(v[:], zim, mybir.ActivationFunctionType.Square)
                nc.scalar.copy(w_[:], zim)
                nc.vector.tensor_sub(ssb_re[:, rs], u[:], v[:])
                nc.vector.tensor_mul(ssb_im[:, rs], zre, w_[:])

            def stage_c(g):
                ps = pp.tile([P, 1024], F32, space="PSUM", tag="ps")
                for bb in range(4):
                    b_ = g * 4 + bb
                    sl = ps[:, bb * 256:(bb + 1) * 256]
                    lre = ssb_re[:, b_ * P:(b_ + 1) * P]
                    lim = ssb_im[:, b_ * P:(b_ + 1) * P]
                    nc.tensor.matmul(out=sl, lhsT=lre, rhs=wi1_s[:], start=True, stop=False)
                    nc.tensor.matmul(out=sl, lhsT=lim, rhs=wi2_s[:], start=False, stop=True)
                preC = ps[:].rearrange("p (b2 h k) -> p b2 h k", b2=4, h=2)[:, :, 0, :]
                pimC = ps[:].rearrange("p (b2 h k) -> p b2 h k", b2=4, h=2)[:, :, 1, :]
                n1 = tp.tile([P, 512], F32R, tag="n1")
                n2 = tp.tile([P, 512], F32R, tag="n2")
                n3 = tp.tile([P, 512], F32R, tag="n3")
                n4 = tp.tile([P, 512], F32R, tag="n4")
                nim = tp.tile([P, 512], F32R, tag="nim")
                nc.scalar.copy(nim[:], pimC)
                nc.vector.tensor_mul(r3(n1[:]), preC, r3(t2re_s[:]))
                nc.gpsimd.tensor_mul(r3(n2[:]), r3(nim[:]), r3(t2imn_s[:]))
                nc.vector.tensor_mul(r3(n3[:]), preC, r3(t2im_s[:]))
                nc.vector.tensor_mul(r3(n4[:]), pimC, r3(t2re_s[:]))
                nt_[g] = (n1, n2, n3, n4)

            def stage_d(g):
                n1, n2, n3, n4 = nt_.pop(g)
                rs = slice(g * 512, (g + 1) * 512)
                ps = pp.tile([P, 1024], F32, space="PSUM", tag="ps")
                po = ps[:, 0:512]
                nc.tensor.matmul(out=po, lhsT=wiim_s[:], rhs=n1[:], start=True, stop=False)
                nc.tensor.matmul(out=po, lhsT=wiim_s[:], rhs=n2[:], start=False, stop=False)
                nc.tensor.matmul(out=po, lhsT=wire_s[:], rhs=n3[:], start=False, stop=False)
                nc.tensor.matmul(out=po, lhsT=wire_s[:], rhs=n4[:], start=False, stop=True)
                nc.scalar.copy(osb[:, rs], po)
                if PER_GROUP_OUT:
                    nc.sync.dma_start(
                        out[:].rearrange("b (a c) -> a b c", c=P)[:, g * 4:(g + 1) * 4, :],
                        osb[:, rs].rearrange("a (b c) -> a b c", c=P))

            for gg in range(11):
                if gg < 8 and not skip_fft:
                    stage_a(gg)
                if 1 <= gg < 9 and not skip_fft:
                    stage_b(gg - 1)
                if 2 <= gg < 10 and not skip_fft:
                    stage_c(gg - 2)
                if 3 <= gg and not skip_fft:
                    stage_d(gg - 3)
            if skip_fft:
                nc.vector.memset(osb[:], 0.0)
            if not PER_GROUP_OUT or skip_fft:
                nc.sync.dma_start(out[:].rearrange("b (a c) -> a b c", c=P),
                                  osb[:].rearrange("a (b c) -> a b c", c=P))

    nc.compile()
    return nc


def _host_consts():
    j = np.arange(P)
    f32 = np.float32
    ang = -2.0 * np.pi * np.outer(j, j) / P
    wf_re, wf_im = np.cos(ang), np.sin(ang)
    wi_re, wi_im = np.cos(-ang), np.sin(-ang)
    wa1 = np.concatenate([wf_re, wf_im], axis=1).astype(f32)
    wa2 = np.concatenate([-wf_im, wf_re], axis=1).astype(f32)
    wi1 = np.concatenate([wi_re, wi_im], axis=1).astype(f32)
    wi2 = np.concatenate([-2.0 * wi_im, 2.0 * wi_re], axis=1).astype(f32)
    tang = -2.0 * np.pi * np.outer(j, j) / O
    t1re_1 = np.cos(tang)
    t1im_1 = np.sin(tang)
    scale = 1.0 / (2.0 * O)
    t2re_1 = np.cos(tang) * scale      # cos(+x) = cos(-x)
    t2im_1 = -np.sin(tang) * scale     # sin(+x) = -sin(-x)

    def b4(m):
        return np.tile(m[:, None, :], (1, 4, 1)).reshape(P, 4 * P).astype(f32)

    return dict(
        wa1=wa1, wa2=wa2, wi1=wi1, wi2=wi2,
        wfre=wf_re.astype(f32), wfim=wf_im.astype(f32), wfimn=(-wf_im).astype(f32),
        wire=wi_re.astype(f32), wiim=wi_im.astype(f32),
        t1re=b4(t1re_1), t1im=b4(t1im_1), t1imn=b4(-t1im_1),
        t2re=b4(t2re_1), t2im=b4(t2im_1), t2imn=b4(-t2im_1),
        iot=np.tile(np.arange(P, dtype=f32)[None, :], (P, 8)),
    )


def _host_prep(h1):
    """Group features by qlow = h1 % 128; slot (q, j) layout with pad 64.

    Returns (slot_pad, slot2d [128, 64] original-d per (q, j) or -1,
    nhv [128, 64] float table of nhigh per slot position)."""
    h1 = np.asarray(h1, dtype=np.int64)
    qlow = h1 % P
    nhigh = h1 // P
    groups = [np.where(qlow == q)[0] for q in range(P)]
    mx = max(g.size for g in groups)
    assert mx <= 64, f"group size {mx} > 64; need 128-slot fallback"
    slot2d = np.full((P, 64), -1, np.int64)
    for q, g in enumerate(groups):
        slot2d[q, :g.size] = g
    # nhv[p, qq]: slot (q = 2*qq + p//64, j = p%64)
    nhv = np.full((P, 64), -1.0, np.float32)
    for p in range(P):
        for qq in range(64):
            d = slot2d[2 * qq + p // 64, p % 64]
            if d >= 0:
                nhv[p, qq] = float(nhigh[d])
    return 64, slot2d, nhv


_last_results = None


def kernel(x1, x2, h1, s1, output_size=O, **kw):
    global _last_results
    x1 = np.asarray(x1, np.float32)
    x2 = np.asarray(x2, np.float32)
    s1 = np.asarray(s1, np.float32)
    slot_pad, slot2d, nhv = _host_prep(h1)
    # sxE2 [128, 64*128] block-diagonal per q-pair: partitions p<64 hold
    # q=2qq (batch cols 0:64), p>=64 hold q=2qq+1 (batch cols 64:128).
    sx = np.concatenate([x1 * s1, x2 * s1], axis=1)  # [B, 2D]: cols = (h, d)
    # value for slot (q, j), batch-col c (c<32: y1, c>=32: y2) = sx-like
    # build full [B=256, 128q, 64j, 64c?] too big; do per core below.
    if slot_pad not in _cache:
        _cache[slot_pad] = _build(slot_pad)
    nc = _cache[slot_pad]
    consts = _host_consts()
    d_of = slot2d  # [128 q, 64 j]
    valid = d_of >= 0
    in_maps = []
    for c in range(NCORES):
        xb1 = x1[c * BC:(c + 1) * BC] * s1    # [32, D]
        xb2 = x2[c * BC:(c + 1) * BC] * s1
        # vals[q, j, b] for b in 0..63 = [y1 32 | y2 32]
        vals = np.zeros((P, 64, 64), np.float32)
        vals[valid, :BC] = xb1[:, d_of[valid]].T
        vals[valid, BC:] = xb2[:, d_of[valid]].T
        sxe = np.zeros((P, 64, P), np.float32)
        qq = np.arange(64)
        # partitions 0:64 <- q even slots; batch cols 0:64
        sxe[:64, :, :64] = vals[2 * qq][:, :, :].transpose(1, 0, 2)
        sxe[64:, :, 64:] = vals[2 * qq + 1][:, :, :].transpose(1, 0, 2)
        m = dict(consts)
        m["sxe"] = sxe.reshape(P, 64 * P)
        m["nhv"] = nhv
        in_maps.append(m)
    res = run_bass_kernel_spmd(nc, in_maps, core_ids=list(range(NCORES)))
    _last_results = res
    return np.concatenate([res.results[c]["out"] for c in range(NCORES)], axis=0)
